# revision 19
# baseline (speedup 1.0000x reference)
"""DIN attention Bass kernel for Trainium2, 8-core data-parallel. v2.

Per core: BL=256 rows, 8 chunks of 32 rows, token compaction (rows globally
sorted by unmasked-token count into 8 bands; band ci = chunk ci on every
core, so all cores share per-chunk width W and the SPMD program).

Math: x@W1 folds to K@(W1b-W1c) + (q*K)@W1d + qb, with qb absorbed into two
fp8 DoubleRow planes via a min-norm solve (plane0 = K+a_b, plane1 = q*K+v_b).
Pad token columns ship ZERO planes, so a pad token scores exactly
s0 = w3.relu(b2); the host subtracts npad*exp(s0+b3) from each row's U.

Performance structure (cost-model driven):
- rhs shipped PLANE-MAJOR [E,2,32,W] fp8 -> W1 is 8 matmuls of ap=4W (DR),
  each into its own PSUM bank (pool of 3 rotating 1-bank tiles).
- relu1 per 4-slot group (8 ops of free 4W), spread over Act/DVE/Pool.
- W2 per t: 2 matmuls (slot-parity merged, ap=4W) into one bank; relu2
  per t (4 ops of free 4W).  W3 per t (ap=4W, block-diag 2-row stationary).
- Three DMA chains: SP ships rhs8+results, Act ships ktm, Pool small stuff.
- One-chunk tail skew: W3/exp/transpose/wcol/wsum of chunk ci-1 are emitted
  interleaved into chunk ci's PE stream to fill relu latency windows.
- Weighted sums: per-row N=1 matmuls (ktm stationary); U via ones column;
  sums+U DMA'd straight from PSUM; host divides, corrects pads, unsorts.
"""

import numpy as np

B, S, E = 2048, 200, 128
H1, H2 = 128, 64
NCORES = 8
BL = B // NCORES          # 256
CHUNK = 32
NCHUNK = BL // CHUNK      # 8

# engine assignment tables (tunable): relu1 t0..t3, relu2 t0..t3
# (GPSIMD cannot access PSUM, so relu ops go on Act/DVE only)
R1_ENG = ["act", "dve", "act", "dve"]
R2_ENG = ["dve", "act", "dve", "act"]

_prog_cache = {}


def _build_program(widths):
    import concourse.bass as bass
    import concourse.mybir as mybir
    import concourse.tile as tile
    from concourse import bacc
    from concourse.masks import make_identity
    from contextlib import ExitStack

    f32 = mybir.dt.float32
    bf16 = mybir.dt.bfloat16
    fp8 = mybir.dt.float8e4
    AF = mybir.ActivationFunctionType
    ALU = mybir.AluOpType
    DR = mybir.MatmulPerfMode.DoubleRow

    nc = bacc.Bacc(None, target_bir_lowering=False, debug=False)

    rhs_d = [nc.declare_dram_parameter(f"rhs{ci}", [E, 2, CHUNK, widths[ci]],
                                       fp8, False) for ci in range(NCHUNK)]
    ktm_d = [nc.declare_dram_parameter(f"ktm{ci}", [widths[ci], CHUNK, E],
                                       bf16, False) for ci in range(NCHUNK)]
    w1dr_d = nc.declare_dram_parameter("w1dr", [E, 2, H1], fp8, False)
    w2_d = nc.declare_dram_parameter("w2", [H1, H2], bf16, False)
    w3s_d = nc.declare_dram_parameter("w3s", [2 * H2, 32], bf16, False)
    b2s_d = nc.declare_dram_parameter("b2s", [2 * H2, 1], f32, False)
    b3v_d = nc.declare_dram_parameter("b3v", [1, 1], f32, False)
    out_d = nc.declare_dram_parameter("out", [E, NCHUNK * 2 * CHUNK], f32, True)

    with tile.TileContext(nc) as tc, ExitStack() as ctx:
        const = ctx.enter_context(tc.tile_pool(name="const", bufs=1))
        kpool = ctx.enter_context(tc.tile_pool(name="keys", bufs=1))
        work = ctx.enter_context(tc.tile_pool(name="work", bufs=2))
        spool = ctx.enter_context(tc.tile_pool(name="smax", bufs=2))
        ps1p = ctx.enter_context(tc.tile_pool(name="ps1", bufs=2, space="PSUM"))
        ps2p = ctx.enter_context(tc.tile_pool(name="ps2", bufs=2, space="PSUM"))
        ps3p = ctx.enter_context(tc.tile_pool(name="ps3", bufs=1, space="PSUM"))
        wpsp = ctx.enter_context(tc.tile_pool(name="wps", bufs=1, space="PSUM"))

        # ---- startup: critical-path first.  SP: w1dr + rhs chain.
        # Pool: rhs0b, then small weights, then the ktm chain. ----
        w1dr = const.tile([E, 2, H1], fp8)
        nc.sync.dma_start(w1dr, w1dr_d[:])
        rhs_t, ktm_t = [], []
        for ci in range(NCHUNK):
            W = widths[ci]
            rhs8 = kpool.tile([E, 2, CHUNK, W], fp8, tag=f"rhs{ci}")
            ktm = kpool.tile([128, CHUNK, E], bf16, tag=f"ktm{ci}")
            rhs_t.append(rhs8)
            ktm_t.append(ktm)
        nc.sync.dma_start(rhs_t[0][:, :, 0:16], rhs_d[0][:, :, 0:16])
        nc.gpsimd.dma_start(rhs_t[0][:, :, 16:32], rhs_d[0][:, :, 16:32])
        w2c = const.tile([H1, H2], bf16)
        nc.sync.dma_start(w2c, w2_d[:])
        b2s = const.tile([2 * H2, 1], f32)
        nc.gpsimd.dma_start(b2s, b2s_d[:])
        w3s = const.tile([2 * H2, 32], bf16)
        nc.gpsimd.dma_start(w3s, w3s_d[:])
        b3t = const.tile([128, 1], f32)
        nc.gpsimd.dma_start(b3t, b3v_d[:].to_broadcast((128, 1)))
        for ci in range(1, NCHUNK):
            nc.sync.dma_start(rhs_t[ci], rhs_d[ci][:])
        for ci in range(NCHUNK):
            nc.gpsimd.dma_start(ktm_t[ci][0:widths[ci]], ktm_d[ci][:])
        ident_bf = const.tile([128, 128], bf16)
        make_identity(nc, ident_bf)
        ones_bf = const.tile([128, 1], bf16)
        nc.vector.memset(ones_bf, 1.0)
        outbuf = const.tile([E, NCHUNK, 2 * CHUNK], f32)
        nc.vector.memset(outbuf, 0.0)

        st = {}

        def vec_op(eng, dst, src, bias=None):
            """relu (+optional bias) on the chosen engine."""
            if eng == "act":
                if bias is None:
                    nc.scalar.activation(dst, src, AF.Relu)
                else:
                    nc.scalar.activation(dst, src, AF.Relu, bias=bias)
            elif eng == "dve":
                if bias is None:
                    nc.vector.tensor_scalar(dst, src, 0.0, None, ALU.max)
                else:
                    nc.vector.tensor_scalar(dst, src, bias, 0.0,
                                            ALU.add, ALU.max)
            else:
                if bias is None:
                    nc.gpsimd.tensor_scalar(dst, src, 0.0, None, ALU.max)
                else:
                    nc.gpsimd.tensor_scalar(dst, src, bias, 0.0,
                                            ALU.add, ALU.max)

        def emit_w1(ci, t):
            """W1 for t (8 slots): two DR matmuls into a 2-bank tile."""
            W = widths[ci]
            ps1 = ps1p.tile([128, 2, 512], f32, tag="ps1",
                            name=f"ps1_{ci}_{t}")
            st[(ci, "ps1", t)] = ps1
            for g in range(2):
                s0 = 8 * t + 4 * g
                nc.tensor.matmul(ps1[:, g, 0:4 * W], w1dr,
                                 rhs_t[ci][:, :, s0:s0 + 4, :],
                                 start=True, stop=True, perf_mode=DR,
                                 skip_group_check=True)

        def emit_relu1(ci, t):
            W = widths[ci]
            ps1 = st[(ci, "ps1", t)]
            h1 = work.tile([128, 8, W], bf16, tag=f"h1_{t}",
                           name=f"h1_{ci}_{t}")
            st[(ci, "h1", t)] = h1
            src = ps1[:, :, 0:4 * W].rearrange("p g (j w) -> p g j w", w=W)
            vec_op(R1_ENG[t], h1.rearrange("p (g j) w -> p g j w", g=2), src)

        def emit_w2(ci, t):
            """2 matmuls (slot parity r) into ps2t[64r:, 0:4W]."""
            W = widths[ci]
            ps2 = ps2p.tile([128, 512], f32, tag="ps2", name=f"ps2_{ci}_{t}")
            st[(ci, "ps2", t)] = ps2
            h1 = st[(ci, "h1", t)]
            for r in range(2):
                mov = h1.rearrange("p (c r) w -> p r c w", r=2)[:, r]
                nc.tensor.matmul(ps2[64 * r:64 * r + 64, 0:4 * W], w2c, mov,
                                 start=True, stop=True,
                                 tile_position=(0, 64 * r),
                                 skip_group_check=True)

        def emit_relu2(ci, t):
            W = widths[ci]
            ps2 = st[(ci, "ps2", t)]
            h2 = work.tile([128, 4, W], bf16, tag=f"h2_{t}", name=f"h2_{ci}_{t}")
            st[(ci, "h2", t)] = h2
            src = ps2[:, 0:4 * W].rearrange("p (c w) -> p c w", w=W)
            vec_op(R2_ENG[t], h2, src, bias=b2s[:, 0:1])

        def emit_w3(ci, j):
            W = widths[ci]
            if j == 0:
                st[(ci, "ps3")] = ps3p.tile([128, 512], f32, tag="ps3", name=f"ps3_{ci}")
            ps3 = st[(ci, "ps3")]
            h2 = st[(ci, "h2", j)]
            nc.tensor.matmul(ps3[32 * j:32 * j + 32, 0:4 * W], w3s,
                             h2.rearrange("p a b -> p (a b)"),
                             start=True, stop=True, tile_position=(0, 32 * j))

        def emit_exp(ci):
            W = widths[ci]
            ps3 = st[(ci, "ps3")]
            u_sp = spool.tile([98, 4, W], bf16, tag="usp", name=f"usp_{ci}")
            st[(ci, "usp")] = u_sp
            nc.scalar.activation(
                u_sp, ps3[0:98, 0:4 * W].rearrange("p (a b) -> p a b", b=W),
                AF.Exp, bias=b3t[0:98, 0:1])

        def emit_transp(ci):
            W = widths[ci]
            u_sp = st[(ci, "usp")]
            wps = wpsp.tile([128, 4, 128], bf16, tag="wps", name=f"wps_{ci}")
            st[(ci, "wps")] = wps
            for qq in range(4):
                nc.tensor.transpose(wps[0:W, qq, 0:98], u_sp[:, qq, 0:W],
                                    ident_bf[0:98, 0:98])

        def emit_wcol(ci):
            W = widths[ci]
            wps = st[(ci, "wps")]
            wcol = spool.tile([128, 4, 4, 2], bf16, tag="wcol", name=f"wcol_{ci}")
            st[(ci, "wcol")] = wcol
            wv = wps.rearrange("p q (j x) -> p q j x", x=32)
            nc.vector.tensor_copy(wcol[0:W], wv[0:W, :, :, 0:2])

        def emit_wsum(ci, half):
            W = widths[ci]
            wcol = st[(ci, "wcol")]
            ktm = ktm_t[ci]
            pso = st[(ci, "ps3")][:, 448:512]
            for slot in range(16 * half, 16 * half + 16):
                j, qq, r = slot // 8, (slot % 8) // 2, slot % 2
                wc = wcol[0:W, qq, j, r:r + 1]
                nc.tensor.matmul(pso[:, slot:slot + 1], ktm[0:W, slot, :], wc,
                                 start=True, stop=True)
                nc.tensor.matmul(pso[0:1, CHUNK + slot:CHUNK + slot + 1],
                                 ones_bf[0:W, 0:1], wc,
                                 start=True, stop=True)

        def emit_out(ci):
            pso = st[(ci, "ps3")][:, 448:512]
            nc.vector.tensor_copy(outbuf[:, ci, 0:CHUNK], pso[:, 0:CHUNK])
            nc.vector.tensor_copy(outbuf[0:1, ci, CHUNK:2 * CHUNK],
                                  pso[0:1, CHUNK:2 * CHUNK])

        # ---------------- main software-pipelined loop ----------------
        for ci in range(NCHUNK):
            pv = ci - 1
            emit_w1(ci, 0)
            emit_relu1(ci, 0)
            emit_w1(ci, 1)
            emit_relu1(ci, 1)
            if pv >= 0:
                for j in range(4):
                    emit_w3(pv, j)
                emit_exp(pv)
            emit_w1(ci, 2)
            emit_relu1(ci, 2)
            emit_w1(ci, 3)
            emit_relu1(ci, 3)
            emit_w2(ci, 0)
            emit_relu2(ci, 0)
            emit_w2(ci, 1)
            emit_relu2(ci, 1)
            emit_w2(ci, 2)
            emit_relu2(ci, 2)
            if pv >= 0:
                emit_transp(pv)
                emit_wcol(pv)
            emit_w2(ci, 3)
            emit_relu2(ci, 3)
            if pv >= 0:
                emit_wsum(pv, 0)
                emit_wsum(pv, 1)
                emit_out(pv)
        nc.sync.dma_start(out_d[:, 0:6 * 2 * CHUNK],
                          outbuf[:, 0:6].rearrange("p a b -> p (a b)"))
        last = NCHUNK - 1
        for j in range(4):
            emit_w3(last, j)
        emit_exp(last)
        emit_transp(last)
        emit_wcol(last)
        emit_wsum(last, 0)
        emit_wsum(last, 1)
        emit_out(last)
        nc.sync.dma_start(out_d[:, 6 * 2 * CHUNK:],
                          outbuf[:, 6:].rearrange("p a b -> p (a b)"))
    nc.compile()
    return nc


def _host_prep(querys, keys, W1, b1, W2, b2, W3, b3, mask):
    import ml_dtypes
    bf = ml_dtypes.bfloat16
    f8 = ml_dtypes.float8_e4m3
    q = np.ascontiguousarray(querys[:, 0, :], dtype=np.float32)   # [B, E]
    W1a, W1b, W1c, W1d = W1[0:128], W1[128:256], W1[256:384], W1[384:512]
    W1bc = (W1b - W1c).astype(np.float32)
    qb = q @ (W1a + W1c) + b1                                      # [B, H1]
    # min-norm absorption of qb into the two DoubleRow planes
    A = np.concatenate([W1bc.T, W1d.T], axis=1)                    # [128, 256]
    av = (A.T @ np.linalg.solve(A @ A.T, qb.T)).T                  # [B, 256]
    a_b, v_b = av[:, :128], av[:, 128:]

    counts = mask.sum(axis=1).astype(np.int64)                     # [B]
    assert counts.max() <= 128, f"token count {counts.max()} > 128 unsupported"
    order = np.argsort(counts, kind="stable")[::-1]                # descending
    widths = []
    for ci in range(NCHUNK):
        band = order[ci * NCORES * CHUNK:(ci + 1) * NCORES * CHUNK]
        widths.append(max(8, int(-(-counts[band].max() // 8) * 8)))
    widths = tuple(int(w) for w in widths)
    assign = order.reshape(NCHUNK, NCORES, CHUNK)

    rhs_arrs = [[] for _ in range(NCORES)]
    ktm_arrs = [[] for _ in range(NCORES)]
    npad = np.zeros((NCORES, NCHUNK, CHUNK), np.float32)
    for ci in range(NCHUNK):
        W = widths[ci]
        for c in range(NCORES):
            rows = assign[ci, c]                                   # [32]
            Kg = np.zeros((CHUNK, W, E), np.float32)
            live = np.zeros((CHUNK, W, 1), np.float32)
            for s_i, r_i in enumerate(rows):
                toks = np.nonzero(mask[r_i])[0]
                Kg[s_i, :len(toks)] = keys[r_i, toks]
                live[s_i, :len(toks)] = 1.0
                npad[c, ci, s_i] = W - len(toks)
            p0 = (Kg + a_b[rows][:, None, :]) * live
            p1 = (Kg * q[rows][:, None, :] + v_b[rows][:, None, :]) * live
            rhs = np.stack([p0, p1], axis=0)                       # [2,32,W,E]
            rhs_arrs[c].append(np.ascontiguousarray(
                rhs.transpose(3, 0, 1, 2)).astype(f8))             # [E,2,32,W]
            ktm_arrs[c].append(np.ascontiguousarray(
                Kg.transpose(1, 0, 2)).astype(bf))                 # [W,32,E]

    w1dr = np.ascontiguousarray(
        np.stack([W1bc, W1d], axis=1)).astype(f8)                  # [E,2,H1]
    w3s = np.zeros((2 * H2, 32), bf)
    w3s[0:H2, 0] = W3[:, 0].astype(bf)
    w3s[H2:, 1] = W3[:, 0].astype(bf)
    b2s = np.concatenate([b2, b2]).reshape(2 * H2, 1).astype(np.float32)
    # pad-token exp value, matching device arithmetic (bf16 h2, bf16 w3)
    h2pad = np.maximum(b2, 0.0).astype(bf).astype(np.float32)
    w3b = W3[:, 0].astype(bf).astype(np.float32)
    s0 = float(h2pad @ w3b)
    u_pad = float(np.float32(
        np.exp(np.float32(s0 + float(np.ravel(b3)[0])))).astype(bf))
    return dict(widths=widths, assign=assign, rhs=rhs_arrs, ktm=ktm_arrs,
                npad=npad, u_pad=u_pad, w1dr=w1dr,
                w2=W2.astype(bf), w3s=w3s, b2s=b2s,
                b3v=np.asarray(b3, np.float32).reshape(1, 1))


def kernel(querys, keys, W1, b1, W2, b2, W3, b3, mask):
    from concourse.bass_utils import run_bass_kernel_spmd

    querys = np.asarray(querys, dtype=np.float32)
    keys = np.asarray(keys, dtype=np.float32)
    W1 = np.asarray(W1, dtype=np.float32)
    b1 = np.asarray(b1, dtype=np.float32)
    W2 = np.asarray(W2, dtype=np.float32)
    b2 = np.asarray(b2, dtype=np.float32)
    W3 = np.asarray(W3, dtype=np.float32)
    b3 = np.asarray(b3, dtype=np.float32)
    mask = np.asarray(mask)
    hp = _host_prep(querys, keys, W1, b1, W2, b2, W3, b3, mask)

    widths = hp["widths"]
    if widths not in _prog_cache:
        _prog_cache[widths] = _build_program(widths)
    prog = _prog_cache[widths]

    in_maps = []
    for c in range(NCORES):
        m = {f"rhs{ci}": hp["rhs"][c][ci] for ci in range(NCHUNK)}
        m.update({f"ktm{ci}": hp["ktm"][c][ci] for ci in range(NCHUNK)})
        m.update({"w1dr": hp["w1dr"], "w2": hp["w2"],
                  "w3s": hp["w3s"], "b2s": hp["b2s"], "b3v": hp["b3v"]})
        in_maps.append(m)

    res = run_bass_kernel_spmd(prog, in_maps, list(range(NCORES)))
    out = np.empty((B, E), np.float32)
    assign = hp["assign"]
    for c in range(NCORES):
        o = res.results[c]["out"].reshape(E, NCHUNK, 2 * CHUNK)
        for ci in range(NCHUNK):
            sums = o[:, ci, 0:CHUNK]                  # [E, 32]
            U = o[0, ci, CHUNK:2 * CHUNK]             # [32]
            U = U - hp["npad"][c, ci] * hp["u_pad"]
            out[assign[ci, c]] = (sums / U[None, :]).T
    return out


# revision 20
# speedup vs baseline: 1.3116x; 1.3116x over previous
"""DIN attention Bass kernel for Trainium2, 8-core data-parallel. v2.

Per core: BL=256 rows, 8 chunks of 32 rows, token compaction (rows globally
sorted by unmasked-token count into 8 bands; band ci = chunk ci on every
core, so all cores share per-chunk width W and the SPMD program).

Math: x@W1 folds to K@(W1b-W1c) + (q*K)@W1d + qb, with qb absorbed into two
fp8 DoubleRow planes via a min-norm solve (plane0 = K+a_b, plane1 = q*K+v_b).
Pad token columns ship ZERO planes, so a pad token scores exactly
s0 = w3.relu(b2); the host subtracts npad*exp(s0+b3) from each row's U.

Performance structure (cost-model driven):
- rhs shipped PLANE-MAJOR [E,2,32,W] fp8 -> W1 is 8 matmuls of ap=4W (DR),
  each into its own PSUM bank (pool of 3 rotating 1-bank tiles).
- relu1 per 4-slot group (8 ops of free 4W), spread over Act/DVE/Pool.
- W2 per t: 2 matmuls (slot-parity merged, ap=4W) into one bank; relu2
  per t (4 ops of free 4W).  W3 per t (ap=4W, block-diag 2-row stationary).
- Three DMA chains: SP ships rhs8+results, Act ships ktm, Pool small stuff.
- One-chunk tail skew: W3/exp/transpose/wcol/wsum of chunk ci-1 are emitted
  interleaved into chunk ci's PE stream to fill relu latency windows.
- Weighted sums: per-row N=1 matmuls (ktm stationary); U via ones column;
  sums+U DMA'd straight from PSUM; host divides, corrects pads, unsorts.
"""

import numpy as np

B, S, E = 2048, 200, 128
H1, H2 = 128, 64
NCORES = 8
BL = B // NCORES          # 256
CHUNK = 32
NCHUNK = BL // CHUNK      # 8

# engine assignment tables (tunable): relu1 t0..t3, relu2 t0..t3
# (GPSIMD cannot access PSUM, so relu ops go on Act/DVE only)
R1_ENG = ["act", "dve", "act", "dve"]
R2_ENG = ["dve", "act", "dve", "act"]

_prog_cache = {}


def _build_program(widths):
    import concourse.bass as bass
    import concourse.mybir as mybir
    import concourse.tile as tile
    from concourse import bacc
    from concourse.masks import make_identity
    from contextlib import ExitStack

    f32 = mybir.dt.float32
    bf16 = mybir.dt.bfloat16
    fp8 = mybir.dt.float8e4
    AF = mybir.ActivationFunctionType
    ALU = mybir.AluOpType
    DR = mybir.MatmulPerfMode.DoubleRow

    nc = bacc.Bacc(None, target_bir_lowering=False, debug=False)

    rhs_d = [nc.declare_dram_parameter(f"rhs{ci}", [E, 2, CHUNK, widths[ci]],
                                       fp8, False) for ci in range(NCHUNK)]
    ktm_d = [nc.declare_dram_parameter(f"ktm{ci}", [widths[ci], CHUNK, E],
                                       bf16, False) for ci in range(NCHUNK)]
    w1dr_d = nc.declare_dram_parameter("w1dr", [E, 2, H1], fp8, False)
    w2_d = nc.declare_dram_parameter("w2", [H1, H2], bf16, False)
    w3s_d = nc.declare_dram_parameter("w3s", [2 * H2, 32], bf16, False)
    b2s_d = nc.declare_dram_parameter("b2s", [2 * H2, 1], f32, False)
    b3v_d = nc.declare_dram_parameter("b3v", [1, 1], f32, False)
    out_d = nc.declare_dram_parameter("out", [E, NCHUNK * 2 * CHUNK], f32, True)

    with tile.TileContext(nc) as tc, ExitStack() as ctx:
        const = ctx.enter_context(tc.tile_pool(name="const", bufs=1))
        kpool = ctx.enter_context(tc.tile_pool(name="keys", bufs=1))
        work = ctx.enter_context(tc.tile_pool(name="work", bufs=2))
        spool = ctx.enter_context(tc.tile_pool(name="smax", bufs=2))
        ps1p = ctx.enter_context(tc.tile_pool(name="ps1", bufs=2, space="PSUM"))
        ps2p = ctx.enter_context(tc.tile_pool(name="ps2", bufs=2, space="PSUM"))
        ps3p = ctx.enter_context(tc.tile_pool(name="ps3", bufs=1, space="PSUM"))
        wpsp = ctx.enter_context(tc.tile_pool(name="wps", bufs=1, space="PSUM"))

        # ---- startup: critical-path first.  SP: w1dr + rhs chain.
        # Pool: rhs0b, then small weights, then the ktm chain. ----
        w1dr = const.tile([E, 2, H1], fp8)
        nc.sync.dma_start(w1dr, w1dr_d[:])
        rhs_t, ktm_t = [], []
        for ci in range(NCHUNK):
            W = widths[ci]
            rhs8 = kpool.tile([E, 2, CHUNK, W], fp8, tag=f"rhs{ci}")
            ktm = kpool.tile([128, CHUNK, E], bf16, tag=f"ktm{ci}")
            rhs_t.append(rhs8)
            ktm_t.append(ktm)
        nc.sync.dma_start(rhs_t[0][:, :, 0:16], rhs_d[0][:, :, 0:16])
        nc.gpsimd.dma_start(rhs_t[0][:, :, 16:32], rhs_d[0][:, :, 16:32])
        w2c = const.tile([H1, H2], bf16)
        nc.sync.dma_start(w2c, w2_d[:])
        b2s = const.tile([2 * H2, 1], f32)
        nc.gpsimd.dma_start(b2s, b2s_d[:])
        w3s = const.tile([2 * H2, 32], bf16)
        nc.gpsimd.dma_start(w3s, w3s_d[:])
        b3t = const.tile([128, 1], f32)
        nc.gpsimd.dma_start(b3t, b3v_d[:].to_broadcast((128, 1)))
        ident_bf = const.tile([128, 128], bf16)
        make_identity(nc, ident_bf)
        ones_bf = const.tile([128, 1], bf16)
        nc.vector.memset(ones_bf, 1.0)
        outbuf = const.tile([E, NCHUNK, 2 * CHUNK], f32)
        nc.vector.memset(outbuf, 0.0)
        for ci in range(1, NCHUNK):
            nc.sync.dma_start(rhs_t[ci], rhs_d[ci][:])
        for ci in range(NCHUNK):
            nc.gpsimd.dma_start(ktm_t[ci][0:widths[ci]], ktm_d[ci][:])

        st = {}

        def vec_op(eng, dst, src, bias=None):
            """relu (+optional bias) on the chosen engine."""
            if eng == "act":
                if bias is None:
                    nc.scalar.activation(dst, src, AF.Relu)
                else:
                    nc.scalar.activation(dst, src, AF.Relu, bias=bias)
            elif eng == "dve":
                if bias is None:
                    nc.vector.tensor_scalar(dst, src, 0.0, None, ALU.max)
                else:
                    nc.vector.tensor_scalar(dst, src, bias, 0.0,
                                            ALU.add, ALU.max)
            else:
                if bias is None:
                    nc.gpsimd.tensor_scalar(dst, src, 0.0, None, ALU.max)
                else:
                    nc.gpsimd.tensor_scalar(dst, src, bias, 0.0,
                                            ALU.add, ALU.max)

        def emit_w1(ci, t):
            """W1 for t (8 slots): two DR matmuls into a 2-bank tile."""
            W = widths[ci]
            ps1 = ps1p.tile([128, 2, 512], f32, tag="ps1",
                            name=f"ps1_{ci}_{t}")
            st[(ci, "ps1", t)] = ps1
            for g in range(2):
                s0 = 8 * t + 4 * g
                nc.tensor.matmul(ps1[:, g, 0:4 * W], w1dr,
                                 rhs_t[ci][:, :, s0:s0 + 4, :],
                                 start=True, stop=True, perf_mode=DR,
                                 skip_group_check=True)

        def emit_relu1(ci, t):
            W = widths[ci]
            ps1 = st[(ci, "ps1", t)]
            h1 = work.tile([128, 8, W], bf16, tag=f"h1_{t}",
                           name=f"h1_{ci}_{t}")
            st[(ci, "h1", t)] = h1
            src = ps1[:, :, 0:4 * W].rearrange("p g (j w) -> p g j w", w=W)
            vec_op(R1_ENG[t], h1.rearrange("p (g j) w -> p g j w", g=2), src)

        def emit_w2(ci, t):
            """2 matmuls (slot parity r) into ps2t[64r:, 0:4W]."""
            W = widths[ci]
            ps2 = ps2p.tile([128, 512], f32, tag="ps2", name=f"ps2_{ci}_{t}")
            st[(ci, "ps2", t)] = ps2
            h1 = st[(ci, "h1", t)]
            for r in range(2):
                mov = h1.rearrange("p (c r) w -> p r c w", r=2)[:, r]
                nc.tensor.matmul(ps2[64 * r:64 * r + 64, 0:4 * W], w2c, mov,
                                 start=True, stop=True,
                                 tile_position=(0, 64 * r),
                                 skip_group_check=True)

        def emit_relu2(ci, t):
            W = widths[ci]
            ps2 = st[(ci, "ps2", t)]
            h2 = work.tile([128, 4, W], bf16, tag=f"h2_{t}", name=f"h2_{ci}_{t}")
            st[(ci, "h2", t)] = h2
            src = ps2[:, 0:4 * W].rearrange("p (c w) -> p c w", w=W)
            vec_op(R2_ENG[t], h2, src, bias=b2s[:, 0:1])

        def emit_w3(ci, j):
            W = widths[ci]
            if j == 0:
                st[(ci, "ps3")] = ps3p.tile([128, 512], f32, tag="ps3", name=f"ps3_{ci}")
            ps3 = st[(ci, "ps3")]
            h2 = st[(ci, "h2", j)]
            nc.tensor.matmul(ps3[32 * j:32 * j + 32, 0:4 * W], w3s,
                             h2.rearrange("p a b -> p (a b)"),
                             start=True, stop=True, tile_position=(0, 32 * j))

        def emit_exp(ci):
            W = widths[ci]
            ps3 = st[(ci, "ps3")]
            u_sp = spool.tile([98, 4, W], bf16, tag="usp", name=f"usp_{ci}")
            st[(ci, "usp")] = u_sp
            nc.scalar.activation(
                u_sp, ps3[0:98, 0:4 * W].rearrange("p (a b) -> p a b", b=W),
                AF.Exp, bias=b3t[0:98, 0:1])

        def emit_transp(ci):
            W = widths[ci]
            u_sp = st[(ci, "usp")]
            wps = wpsp.tile([128, 4, 128], bf16, tag="wps", name=f"wps_{ci}")
            st[(ci, "wps")] = wps
            for qq in range(4):
                nc.tensor.transpose(wps[0:W, qq, 0:98], u_sp[:, qq, 0:W],
                                    ident_bf[0:98, 0:98])

        def emit_wcol(ci):
            W = widths[ci]
            wps = st[(ci, "wps")]
            wcol = spool.tile([128, 4, 4, 2], bf16, tag="wcol", name=f"wcol_{ci}")
            st[(ci, "wcol")] = wcol
            wv = wps.rearrange("p q (j x) -> p q j x", x=32)
            nc.vector.tensor_copy(wcol[0:W], wv[0:W, :, :, 0:2])

        def emit_wsum(ci, half):
            W = widths[ci]
            wcol = st[(ci, "wcol")]
            ktm = ktm_t[ci]
            pso = st[(ci, "ps3")][:, 448:512]
            for slot in range(16 * half, 16 * half + 16):
                j, qq, r = slot // 8, (slot % 8) // 2, slot % 2
                wc = wcol[0:W, qq, j, r:r + 1]
                nc.tensor.matmul(pso[:, slot:slot + 1], ktm[0:W, slot, :], wc,
                                 start=True, stop=True)
                nc.tensor.matmul(pso[0:1, CHUNK + slot:CHUNK + slot + 1],
                                 ones_bf[0:W, 0:1], wc,
                                 start=True, stop=True)

        def emit_out(ci):
            pso = st[(ci, "ps3")][:, 448:512]
            nc.vector.tensor_copy(outbuf[:, ci, 0:CHUNK], pso[:, 0:CHUNK])
            nc.vector.tensor_copy(outbuf[0:1, ci, CHUNK:2 * CHUNK],
                                  pso[0:1, CHUNK:2 * CHUNK])

        # ---------------- main software-pipelined loop ----------------
        for ci in range(NCHUNK):
            pv = ci - 1
            emit_w1(ci, 0)
            emit_relu1(ci, 0)
            emit_w1(ci, 1)
            emit_relu1(ci, 1)
            if pv >= 0:
                for j in range(4):
                    emit_w3(pv, j)
                emit_exp(pv)
            emit_w1(ci, 2)
            emit_relu1(ci, 2)
            emit_w1(ci, 3)
            emit_relu1(ci, 3)
            emit_w2(ci, 0)
            emit_relu2(ci, 0)
            emit_w2(ci, 1)
            emit_relu2(ci, 1)
            emit_w2(ci, 2)
            emit_relu2(ci, 2)
            if pv >= 0:
                emit_transp(pv)
                emit_wcol(pv)
            emit_w2(ci, 3)
            emit_relu2(ci, 3)
            if pv >= 0:
                emit_wsum(pv, 0)
                emit_wsum(pv, 1)
                emit_out(pv)
        nc.sync.dma_start(out_d[:, 0:6 * 2 * CHUNK],
                          outbuf[:, 0:6].rearrange("p a b -> p (a b)"))
        last = NCHUNK - 1
        for j in range(4):
            emit_w3(last, j)
        emit_exp(last)
        emit_transp(last)
        emit_wcol(last)
        emit_wsum(last, 0)
        emit_wsum(last, 1)
        emit_out(last)
        nc.sync.dma_start(out_d[:, 6 * 2 * CHUNK:],
                          outbuf[:, 6:].rearrange("p a b -> p (a b)"))
    nc.compile()
    return nc


def _host_prep(querys, keys, W1, b1, W2, b2, W3, b3, mask):
    import ml_dtypes
    bf = ml_dtypes.bfloat16
    f8 = ml_dtypes.float8_e4m3
    q = np.ascontiguousarray(querys[:, 0, :], dtype=np.float32)   # [B, E]
    W1a, W1b, W1c, W1d = W1[0:128], W1[128:256], W1[256:384], W1[384:512]
    W1bc = (W1b - W1c).astype(np.float32)
    qb = q @ (W1a + W1c) + b1                                      # [B, H1]
    # min-norm absorption of qb into the two DoubleRow planes
    A = np.concatenate([W1bc.T, W1d.T], axis=1)                    # [128, 256]
    av = (A.T @ np.linalg.solve(A @ A.T, qb.T)).T                  # [B, 256]
    a_b, v_b = av[:, :128], av[:, 128:]

    counts = mask.sum(axis=1).astype(np.int64)                     # [B]
    assert counts.max() <= 128, f"token count {counts.max()} > 128 unsupported"
    order = np.argsort(counts, kind="stable")[::-1]                # descending
    widths = []
    for ci in range(NCHUNK):
        band = order[ci * NCORES * CHUNK:(ci + 1) * NCORES * CHUNK]
        widths.append(max(8, int(-(-counts[band].max() // 8) * 8)))
    widths = tuple(int(w) for w in widths)
    assign = order.reshape(NCHUNK, NCORES, CHUNK)

    rhs_arrs = [[] for _ in range(NCORES)]
    ktm_arrs = [[] for _ in range(NCORES)]
    npad = np.zeros((NCORES, NCHUNK, CHUNK), np.float32)
    for ci in range(NCHUNK):
        W = widths[ci]
        for c in range(NCORES):
            rows = assign[ci, c]                                   # [32]
            Kg = np.zeros((CHUNK, W, E), np.float32)
            live = np.zeros((CHUNK, W, 1), np.float32)
            for s_i, r_i in enumerate(rows):
                toks = np.nonzero(mask[r_i])[0]
                Kg[s_i, :len(toks)] = keys[r_i, toks]
                live[s_i, :len(toks)] = 1.0
                npad[c, ci, s_i] = W - len(toks)
            p0 = (Kg + a_b[rows][:, None, :]) * live
            p1 = (Kg * q[rows][:, None, :] + v_b[rows][:, None, :]) * live
            rhs = np.stack([p0, p1], axis=0)                       # [2,32,W,E]
            rhs_arrs[c].append(np.ascontiguousarray(
                rhs.transpose(3, 0, 1, 2)).astype(f8))             # [E,2,32,W]
            ktm_arrs[c].append(np.ascontiguousarray(
                Kg.transpose(1, 0, 2)).astype(bf))                 # [W,32,E]

    w1dr = np.ascontiguousarray(
        np.stack([W1bc, W1d], axis=1)).astype(f8)                  # [E,2,H1]
    w3s = np.zeros((2 * H2, 32), bf)
    w3s[0:H2, 0] = W3[:, 0].astype(bf)
    w3s[H2:, 1] = W3[:, 0].astype(bf)
    b2s = np.concatenate([b2, b2]).reshape(2 * H2, 1).astype(np.float32)
    # pad-token exp value, matching device arithmetic (bf16 h2, bf16 w3)
    h2pad = np.maximum(b2, 0.0).astype(bf).astype(np.float32)
    w3b = W3[:, 0].astype(bf).astype(np.float32)
    s0 = float(h2pad @ w3b)
    u_pad = float(np.float32(
        np.exp(np.float32(s0 + float(np.ravel(b3)[0])))).astype(bf))
    return dict(widths=widths, assign=assign, rhs=rhs_arrs, ktm=ktm_arrs,
                npad=npad, u_pad=u_pad, w1dr=w1dr,
                w2=W2.astype(bf), w3s=w3s, b2s=b2s,
                b3v=np.asarray(b3, np.float32).reshape(1, 1))


def kernel(querys, keys, W1, b1, W2, b2, W3, b3, mask):
    from concourse.bass_utils import run_bass_kernel_spmd

    querys = np.asarray(querys, dtype=np.float32)
    keys = np.asarray(keys, dtype=np.float32)
    W1 = np.asarray(W1, dtype=np.float32)
    b1 = np.asarray(b1, dtype=np.float32)
    W2 = np.asarray(W2, dtype=np.float32)
    b2 = np.asarray(b2, dtype=np.float32)
    W3 = np.asarray(W3, dtype=np.float32)
    b3 = np.asarray(b3, dtype=np.float32)
    mask = np.asarray(mask)
    hp = _host_prep(querys, keys, W1, b1, W2, b2, W3, b3, mask)

    widths = hp["widths"]
    if widths not in _prog_cache:
        _prog_cache[widths] = _build_program(widths)
    prog = _prog_cache[widths]

    in_maps = []
    for c in range(NCORES):
        m = {f"rhs{ci}": hp["rhs"][c][ci] for ci in range(NCHUNK)}
        m.update({f"ktm{ci}": hp["ktm"][c][ci] for ci in range(NCHUNK)})
        m.update({"w1dr": hp["w1dr"], "w2": hp["w2"],
                  "w3s": hp["w3s"], "b2s": hp["b2s"], "b3v": hp["b3v"]})
        in_maps.append(m)

    res = run_bass_kernel_spmd(prog, in_maps, list(range(NCORES)))
    out = np.empty((B, E), np.float32)
    assign = hp["assign"]
    for c in range(NCORES):
        o = res.results[c]["out"].reshape(E, NCHUNK, 2 * CHUNK)
        for ci in range(NCHUNK):
            sums = o[:, ci, 0:CHUNK]                  # [E, 32]
            U = o[0, ci, CHUNK:2 * CHUNK]             # [32]
            U = U - hp["npad"][c, ci] * hp["u_pad"]
            out[assign[ci, c]] = (sums / U[None, :]).T
    return out
